# revision 62
# baseline (speedup 1.0000x reference)
"""Trainium2 Bass kernel for nn_CAModel (neural cellular automaton step).

Computation (per image, fp32):
  pre_life = maxpool3x3(x[...,3]) > 0.1
  gx, gy   = depthwise 3x3 sobel convs of x
  perc     = interleave([x, gx, gy])            # [H,W,48]
  h        = relu(perc @ w0)                    # [H,W,128]
  dx       = h @ w1                             # [H,W,16]
  x_mid    = x + dx * (update_rand <= 0.5)
  life     = pre_life & (maxpool3x3(x_mid[...,3]) > 0.1)
  x_new    = x_mid * life
  returns (x_new, dx)

Mapping: 8 NeuronCores, pure data-parallel over batch (2 images/core).
The 3x3 conv + fc0 are fused into two matmuls (K=96 covering horizontal
taps 0+1 via a column-shifted duplicate of x on partitions 0-47, plus
K=48 for tap 2) with sobel coefficients folded into the weights on the
host.  fc1 runs operand-swapped (h stationary) so dx lands pixel-major.
The elementwise tail runs in a "pixel slab" layout [128 = w%128,
(img,row,half), c]; maxpool uses free-dim shifts vertically and PE
shift-permutation matmuls horizontally.
"""

import functools
import os
import sys

import numpy as np

_TRN_REPO = os.environ.get("TRN_RL_REPO", "/opt/trn_rl_repo")
if _TRN_REPO not in sys.path:
    sys.path.insert(0, _TRN_REPO)

import concourse.bass as bass
import concourse.bacc as bacc
import concourse.tile as tile
from concourse import mybir
from concourse.bass_utils import run_bass_kernel_spmd

F32 = mybir.dt.float32
F32R = mybir.dt.float32r
BF16 = mybir.dt.bfloat16
BF16_NP = mybir.dt.np(mybir.dt.bfloat16)

C = 16          # channels
HID = 128       # hidden dim
PW = 128        # partitions used as w-position within a half
N_CORES = 8
FIRE_RATE = 0.5
ALIVE_THR = 0.1

LAST_RESULTS = None  # BassKernelResults of the most recent kernel() call


# ---------------------------------------------------------------------------
# device program
# ---------------------------------------------------------------------------

def build_program(NI, H, W, TR=16):
    """Build the Bass program for one core processing NI images of HxW."""
    NH = W // PW                  # halves per row
    assert W % PW == 0 and H % TR == 0 and TR % 2 == 0
    Hp, Wp = H + 2, W + 2
    NRH = NI * H * NH             # total (img,row,half) count
    GRP = TR * NH * C             # psum free size per row-tile (=512 for TR=16)
    assert GRP <= 512

    nc = bacc.Bacc(trn_type="TRN2")

    # xq layout [NI, 96, H, Wp]: partition-major with uniform stride H*Wp.
    # Partitions 0-47 hold the column-shifted copy (tap1/tap2 windows),
    # 48-95 the unshifted rows (tap0); dy halo baked in per partition row.
    xh = nc.dram_tensor("xh", [NI * 96 * H, Wp], BF16, kind="ExternalInput")
    x_px = nc.dram_tensor("x_px", [PW, NRH, C], BF16, kind="ExternalInput")
    ur = nc.dram_tensor("ur", [PW, NRH], F32, kind="ExternalInput")
    B2d = nc.dram_tensor("B2d", [96, HID], BF16, kind="ExternalInput")
    B3d = nc.dram_tensor("B3d", [96, HID], BF16, kind="ExternalInput")
    w1d = nc.dram_tensor("w1d", [HID, C], BF16, kind="ExternalInput")
    SEd = nc.dram_tensor("SEd", [PW, PW], BF16, kind="ExternalInput")
    SWd = nc.dram_tensor("SWd", [PW, PW], BF16, kind="ExternalInput")
    dxo = nc.dram_tensor("dxo", [PW, NRH, C], BF16, kind="ExternalOutput")
    xno = nc.dram_tensor("xno", [PW, NRH, C], BF16, kind="ExternalOutput")

    with tile.TileContext(nc) as tc:
        _emit(tc, locals())
    nc.compile()
    return nc


def _emit(tc, t):
    nc = tc.nc
    NI, H, W, TR = t["NI"], t["H"], t["W"], t["TR"]
    NH, Hp, Wp, NRH, GRP = t["NH"], t["Hp"], t["Wp"], t["NRH"], t["GRP"]
    xh, x_px, ur, B2d, B3d, w1d, SEd, SWd, dxo, xno = (
        t["xh"], t["x_px"], t["ur"], t["B2d"], t["B3d"], t["w1d"],
        t["SEd"], t["SWd"], t["dxo"], t["xno"])
    TRNH = TR * NH
    AL = mybir.AluOpType

    from contextlib import ExitStack
    ctx = ExitStack()
    with ctx:
        singles = ctx.enter_context(tc.tile_pool(name="singles", bufs=1))
        xc_pool = ctx.enter_context(tc.tile_pool(name="xc", bufs=3))
        h_pool = ctx.enter_context(tc.tile_pool(name="h", bufs=8))
        st_pool = ctx.enter_context(tc.tile_pool(name="st", bufs=3))
        ps_h = ctx.enter_context(tc.tile_pool(name="ps_h", bufs=4, space="PSUM"))
        ps_dx = ctx.enter_context(tc.tile_pool(name="ps_dx", bufs=2, space="PSUM"))
        ps_scr = ctx.enter_context(tc.tile_pool(name="ps_scr", bufs=1, space="PSUM"))

        # ---- constants / weights ----
        # b2_sb rows 0-47 = tap dx=1 weights, rows 48-95 = tap dx=0 (matches
        # xc2's partition layout: shifted-copy block first).
        b2_sb = singles.tile([96, HID], BF16)
        nc.sync.dma_start(out=b2_sb, in_=B2d.ap())
        # b3 zero-padded to K=96 so both fc0 matmuls use the same (128,128)
        # PE tile config (a 64-row reconfig serializes LDWEIGHTS, +110ns/mm)
        b3_sb = singles.tile([96, HID], BF16)
        nc.sync.dma_start(out=b3_sb, in_=B3d.ap())
        w1_sb = singles.tile([HID, C], BF16)
        nc.sync.dma_start(out=w1_sb, in_=w1d.ap())
        se_sb = singles.tile([PW, PW], BF16)
        nc.sync.dma_start(out=se_sb, in_=SEd.ap())
        sw_sb = singles.tile([PW, PW], BF16)
        nc.sync.dma_start(out=sw_sb, in_=SWd.ap())
        # preload the ACT RELU table so tile 0's relus don't pay it
        wtab = singles.tile([PW, 2], BF16)
        nc.scalar.activation(out=wtab, in_=se_sb[:, 0:2],
                             func=mybir.ActivationFunctionType.Relu)

        # ---- update mask (DMA emitted inside the loop after tile 0's x
        # load so it doesn't delay the first matmul) ----
        ur_sb = singles.tile([PW, NRH], F32)
        um_sb = singles.tile([PW, NRH], F32)

        # ---- PE pre-sync dummies ----
        # Fused 4-byte-weight matmuls (f32/f32r) can carry only one sync
        # wait; touch each DMA-loaded operand once from PE so real matmuls
        # never need more than one.
        scr = ps_scr.tile([PW, 2], F32, tag="scr")
        nc.tensor.matmul(out=scr, lhsT=b2_sb[:], rhs=b2_sb[:, 0:2],
                         start=True, stop=True)
        nc.tensor.matmul(out=scr, lhsT=b3_sb[:], rhs=b3_sb[:, 0:2],
                         start=True, stop=True)
        scr2f = ps_scr.tile([PW, 2], F32, tag="scr")
        scr2 = scr2f[0:C, :]
        nc.tensor.matmul(out=scr2, lhsT=w1_sb[:], rhs=w1_sb[:, 0:2],
                         start=True, stop=True)

        # ---- residents ----
        xmid = singles.tile([PW, NRH, C], F32)
        ax = singles.tile([PW, NRH], BF16)   # alpha of x
        am = singles.tile([PW, NRH], BF16)   # alpha of x_mid

        # ---- mask scratch (bf16; shared across images/parts) ----
        HB = H * NH // 2                      # free size of one part
        vm = singles.tile([PW, NRH], BF16)
        vm2 = singles.tile([PW, NRH], BF16)
        m3 = singles.tile([PW, NRH], BF16)
        plx = singles.tile([PW, NRH], BF16)
        plm = singles.tile([PW, NRH], BF16)
        # cross-half neighbour staging: gather (DVE, strided->contig) into a
        # legal base partition, then a single-descriptor DMA moves it to the
        # partition where the patch max needs it; other partitions stay 0
        NR2 = H // 2
        sge = singles.tile([PW, NR2], BF16)
        nc.vector.memset(sge, 0.0)
        sgw = singles.tile([PW, NR2], BF16)
        nc.vector.memset(sgw, 0.0)

        def emit_masks_and_xnew(img, r0, r1):
            SI = img * H * NH
            S, E = SI + r0 * NH, SI + r1 * NH
            L = E - S

            for alpha, pl in ((ax, plx), (am, plm)):
                # vertical 3-max over rows [r0, r1) (clamped at image rows)
                if r0 == 0:
                    nc.vector.tensor_copy(out=vm[:, S:S + NH],
                                          in_=alpha[:, S:S + NH])
                    nc.vector.tensor_tensor(
                        out=vm[:, S + NH:E], in0=alpha[:, S + NH:E],
                        in1=alpha[:, S:E - NH], op=AL.max)
                else:
                    nc.vector.tensor_tensor(
                        out=vm[:, S:E], in0=alpha[:, S:E],
                        in1=alpha[:, S - NH:E - NH], op=AL.max)
                if r1 == H:
                    nc.vector.tensor_tensor(
                        out=vm2[:, S:E - NH], in0=vm[:, S:E - NH],
                        in1=alpha[:, S + NH:E], op=AL.max)
                    nc.vector.tensor_copy(out=vm2[:, E - NH:E],
                                          in_=vm[:, E - NH:E])
                else:
                    nc.vector.tensor_tensor(
                        out=vm2[:, S:E], in0=vm[:, S:E],
                        in1=alpha[:, S + NH:E + NH], op=AL.max)

                # horizontal 3-max: bf16 PE shift-permutation matmuls (exact
                # copy through the array; border partitions get psum 0)
                psf = ps_scr.tile([PW, 2, L], F32, tag="shift")
                nc.tensor.matmul(out=psf[:, 0, :], lhsT=se_sb[:],
                                 rhs=vm2[:, S:E], start=True, stop=True)
                nc.tensor.matmul(out=psf[:, 1, :], lhsT=sw_sb[:],
                                 rhs=vm2[:, S:E], start=True, stop=True)
                nc.vector.tensor_tensor(out=m3[:, S:E], in0=vm2[:, S:E],
                                        in1=psf[:, 0, :], op=AL.max)
                nc.vector.tensor_tensor(out=m3[:, S:E], in0=m3[:, S:E],
                                        in1=psf[:, 1, :], op=AL.max)
                # cross-half seams (NH=2): east neighbour of (p=127, hf=0)
                # is vm2[0, (r, 1)]; west neighbour of (p=0, hf=1) is
                # vm2[127, (r, 0)]
                def rh(t, p0, pn, hf):
                    return t[p0:p0 + pn, S:E].rearrange(
                        "p (r h) -> p r h", h=NH)[:, :, hf]

                NRr = r1 - r0
                nc.vector.tensor_copy(out=sge[0:1, 0:NRr],
                                      in_=rh(vm2, 0, 1, 1))
                nc.sync.dma_start(out=sge[127:128, 0:NRr],
                                  in_=sge[0:1, 0:NRr])
                nc.vector.tensor_tensor(
                    out=rh(m3, 96, 32, 0), in0=rh(m3, 96, 32, 0),
                    in1=sge[96:128, 0:NRr], op=AL.max)
                nc.vector.tensor_copy(out=sgw[96:128, 0:NRr],
                                      in_=rh(vm2, 96, 32, 0))
                nc.sync.dma_start(out=sgw[0:1, 0:NRr],
                                  in_=sgw[127:128, 0:NRr])
                nc.vector.tensor_tensor(
                    out=rh(m3, 0, 32, 1), in0=rh(m3, 0, 32, 1),
                    in1=sgw[0:32, 0:NRr], op=AL.max)
                nc.vector.tensor_scalar(
                    out=pl[:, S:E], in0=m3[:, S:E], scalar1=ALIVE_THR,
                    scalar2=None, op0=AL.is_gt)

            life = plx  # reuse: life = pre_life * mid_life
            nc.vector.tensor_tensor(out=life[:, S:E], in0=plx[:, S:E],
                                    in1=plm[:, S:E], op=AL.mult)
            for it in range(r0 // TR, r1 // TR):
                g0 = SI + it * TRNH
                xns = st_pool.tile([PW, TRNH, C], BF16, tag="xns")
                nc.vector.tensor_tensor(
                    out=xns, in0=xmid[:, g0:g0 + TRNH, :],
                    in1=life[:, g0:g0 + TRNH, None].to_broadcast([PW, TRNH, C]),
                    op=AL.mult)
                nc.sync.dma_start(out=xno.ap()[:, g0:g0 + TRNH, :], in_=xns,
                                  max_dma_last_dim=256)

        # ================= main matmul + dx/x_mid loop =================
        n_tiles = NI * (H // TR)
        for tt in range(n_tiles):
            img, a = divmod(tt, H // TR)
            a *= TR                       # first real row of this tile
            g0 = (img * H + a) * NH       # first rh index of this tile

            # XC2 partitions 0-47: x shifted one column left (padded col w+1
            # at stored col w); partitions 48-95: unshifted.  Gives taps 0+1
            # as a single K=96 matmul (window 0) and tap 2 as K=48 (window 1
            # into the shifted block) with rhs base partition 0.
            xc2 = xc_pool.tile([96, TR, Wp], BF16)
            # one call, uniform partition stride -> 384 ~2KB descriptors
            # spread across all 16 SDMA engines
            src = bass.AP(
                tensor=xh.ap().tensor,
                offset=(img * 96 * H + a) * Wp,
                ap=[[H * Wp, 96], [Wp, TR], [1, Wp]])
            nc.sync.dma_start(out=xc2, in_=src, max_dma_last_dim=1032)
            if tt == 0:
                nc.sync.dma_start(out=ur_sb, in_=ur.ap(),
                                  max_dma_last_dim=512)
                nc.vector.tensor_scalar(
                    out=um_sb, in0=ur_sb, scalar1=FIRE_RATE, scalar2=None,
                    op0=AL.is_le)

            ps2 = ps_dx.tile([PW, TRNH, C], F32)
            # emit fc0 matmuls pair-grouped by weight (b2 b2 b3 b3) so every
            # LDWEIGHTS has a full 512-cycle matmul to prefetch under, then
            # the relus, then the fc1 matmuls (LDWEIGHTS-bound)
            pshs, hsbs = [], []
            for q in range(TR // 4):
                pshA = ps_h.tile([HID, 2, W], F32, tag="psh0")
                pshB = ps_h.tile([HID, 2, W], F32, tag="psh0")
                pshs += [pshA, pshB]
                pA, pB = 4 * q, 4 * q + 2
                nc.tensor.matmul(
                    out=pshA, lhsT=b2_sb[:], rhs=xc2[0:96, pA:pA + 2, 0:W],
                    start=True, stop=False)
                nc.tensor.matmul(
                    out=pshB, lhsT=b2_sb[:], rhs=xc2[0:96, pB:pB + 2, 0:W],
                    start=True, stop=False)
                nc.tensor.matmul(
                    out=pshA, lhsT=b3_sb[:], rhs=xc2[0:96, pA:pA + 2, 1:1 + W],
                    start=False, stop=True)
                nc.tensor.matmul(
                    out=pshB, lhsT=b3_sb[:], rhs=xc2[0:96, pB:pB + 2, 1:1 + W],
                    start=False, stop=True)
            for p in range(TR // 2):
                hsb = h_pool.tile([HID, 2, W], BF16)
                hsbs.append(hsb)
                if p >= 5:
                    nc.vector.tensor_scalar(
                        out=hsb, in0=pshs[p], scalar1=0.0, scalar2=None,
                        op0=AL.max)
                else:
                    nc.scalar.activation(
                        out=hsb, in_=pshs[p],
                        func=mybir.ActivationFunctionType.Relu)
            for p in range(TR // 2):
                # fc1, operand-swapped: dx[pix, c] chunks
                for rp in range(2):
                    for hf in range(NH):
                        nc.tensor.matmul(
                            out=ps2[:, (2 * p + rp) * NH + hf, :],
                            lhsT=hsbs[p][:, rp, hf * PW:(hf + 1) * PW],
                            rhs=w1_sb[:],
                            start=True, stop=True)

            dxs = st_pool.tile([PW, TRNH, C], BF16, tag="dxs")
            nc.vector.tensor_copy(out=dxs, in_=ps2)
            nc.sync.dma_start(out=dxo.ap()[:, g0:g0 + TRNH, :], in_=dxs,
                              max_dma_last_dim=256)

            xps = st_pool.tile([PW, TRNH, C], BF16, tag="xps")
            nc.sync.dma_start(out=xps, in_=x_px.ap()[:, g0:g0 + TRNH, :],
                              max_dma_last_dim=256)
            nc.gpsimd.tensor_copy(out=ax[:, g0:g0 + TRNH], in_=xps[:, :, 3])

            # x_mid = x + dx * um  (gpsimd: keep DVE free for relu/copies)
            dxm = st_pool.tile([PW, TRNH, C], F32, tag="dxm")
            nc.gpsimd.tensor_tensor(
                out=dxm, in0=dxs,
                in1=um_sb[:, g0:g0 + TRNH, None].to_broadcast([PW, TRNH, C]),
                op=AL.mult)
            nc.gpsimd.tensor_tensor(
                out=xmid[:, g0:g0 + TRNH, :], in0=xps, in1=dxm, op=AL.add)
            nc.gpsimd.tensor_copy(
                out=am[:, g0:g0 + TRNH], in_=xmid[:, g0:g0 + TRNH, 3])

            QH = H // 4
            if a > 0 and a % QH == 0:
                # rows [a-QH, a) are maskable as soon as the alpha of row a
                # exists; overlaps the remaining matmul work
                emit_masks_and_xnew(img, a - QH, a)
            if a + TR == H:
                if img == NI - 1:
                    # last image: two smaller parts so the exposed tail
                    # (nothing left to overlap with) is as short as possible
                    emit_masks_and_xnew(img, H - QH, H - QH // 2)
                    emit_masks_and_xnew(img, H - QH // 2, H)
                else:
                    emit_masks_and_xnew(img, H - QH, H)


def _pslice(tile_, p, hf, NH, hrng):
    """[1, hrng] AP of tile_ at partition p, free elements hf::NH."""
    return tile_[p:p + 1, :].rearrange("p (r h) -> p r h", h=NH)[:, :, hf]


def _prange(tile_, p0, cnt, hf, NH, hrng):
    """[cnt, hrng] AP of tile_ at partitions [p0,p0+cnt), free elems hf::NH."""
    return tile_[p0:p0 + cnt, :].rearrange("p (r h) -> p r h", h=NH)[:, :, hf]


# ---------------------------------------------------------------------------
# host side
# ---------------------------------------------------------------------------

def _sobel():
    kx = np.outer([1.0, 2.0, 1.0], [-1.0, 0.0, 1.0]) / 8.0
    ky = kx.T
    return kx, ky


def make_weights(w0, w1):
    """Fold sobel taps into fc0 -> B2[96,128] = [tap1; tap0], B3[48,128]."""
    kx, ky = _sobel()
    w0 = np.asarray(w0, np.float32)         # [48, 128]
    W0x = w0[0::3]                           # [16, 128]
    W0gx = w0[1::3]
    W0gy = w0[2::3]
    Bw = np.zeros((3, 48, HID), np.float32)  # cast to bf16 at return
    for dy in range(3):
        for dxi in range(3):
            m = kx[dy, dxi] * W0gx + ky[dy, dxi] * W0gy
            if dy == 1 and dxi == 1:
                m = m + W0x
            Bw[dxi, dy * C:(dy + 1) * C, :] = m
    B2 = np.concatenate([Bw[1], Bw[0]], axis=0)
    B3 = np.concatenate([Bw[2], np.zeros((48, HID), np.float32)], axis=0)
    return (B2.astype(BF16_NP), B3.astype(BF16_NP),
            np.asarray(w1, BF16_NP))


def host_inputs(x_core, ur_core, B2, B3, w1, H, W):
    """Build the per-core input map from [NI,H,W,C] x and [NI,H,W,1] rand."""
    NI = x_core.shape[0]
    NH = W // PW
    Hp, Wp = H + 2, W + 2
    # xq[i, dy*16+c + 48, r, :] = xpad[i, c, r+dy, :]; partitions 0-47 are
    # the same rows shifted one column left (zero fill)
    xpad = np.zeros((NI, C, Hp, Wp), BF16_NP)
    xpad[:, :, 1:H + 1, 1:W + 1] = x_core.transpose(0, 3, 1, 2)
    xh = np.zeros((NI, 96, H, Wp), BF16_NP)
    for dy in range(3):
        blk = xpad[:, :, dy:dy + H, :]
        xh[:, 48 + dy * C:48 + (dy + 1) * C] = blk
        xh[:, dy * C:(dy + 1) * C, :, 0:Wp - 1] = blk[..., 1:]
    x_px = np.ascontiguousarray(
        x_core.reshape(NI, H, NH, PW, C).transpose(3, 0, 1, 2, 4)
    ).reshape(PW, NI * H * NH, C).astype(BF16_NP)
    urp = np.ascontiguousarray(
        ur_core[..., 0].reshape(NI, H, NH, PW).transpose(3, 0, 1, 2)
    ).reshape(PW, NI * H * NH)
    return {
        "xh": xh.reshape(NI * 96 * H, Wp),
        "x_px": x_px,
        "ur": urp,
        "B2d": B2,
        "B3d": B3,
        "w1d": w1,
        "SEd": np.eye(PW, k=-1, dtype=np.float32).astype(BF16_NP),
        "SWd": np.eye(PW, k=1, dtype=np.float32).astype(BF16_NP),
    }


def unpack_output(dev, NI, H, W):
    """[PW, NRH, C] device layout -> [NI, H, W, C] float32."""
    NH = W // PW
    return np.ascontiguousarray(
        np.asarray(dev, np.float32).reshape(PW, NI, H, NH, C)
        .transpose(1, 2, 3, 0, 4)
    ).reshape(NI, H, W, C)


@functools.lru_cache(maxsize=2)
def _cached_program(NI, H, W, TR):
    return build_program(NI, H, W, TR=TR)


def kernel(x, update_rand, w0, w1):
    x = np.asarray(x, np.float32)
    update_rand = np.asarray(update_rand, np.float32)
    B, H, W, _ = x.shape
    NI = B // N_CORES
    B2, B3, w1f = make_weights(w0, w1)

    nc = _cached_program(NI, H, W, 16)
    in_maps = [
        host_inputs(x[i * NI:(i + 1) * NI], update_rand[i * NI:(i + 1) * NI],
                    B2, B3, w1f, H, W)
        for i in range(N_CORES)
    ]
    res = run_bass_kernel_spmd(nc, in_maps, core_ids=list(range(N_CORES)))
    global LAST_RESULTS
    LAST_RESULTS = res
    x_new = np.concatenate(
        [unpack_output(r["xno"], NI, H, W) for r in res.results], axis=0)
    dx = np.concatenate(
        [unpack_output(r["dxo"], NI, H, W) for r in res.results], axis=0)
    return x_new, dx



# revision 64
# speedup vs baseline: 1.0153x; 1.0153x over previous
"""Trainium2 Bass kernel for nn_CAModel (neural cellular automaton step).

Computation (per image, fp32):
  pre_life = maxpool3x3(x[...,3]) > 0.1
  gx, gy   = depthwise 3x3 sobel convs of x
  perc     = interleave([x, gx, gy])            # [H,W,48]
  h        = relu(perc @ w0)                    # [H,W,128]
  dx       = h @ w1                             # [H,W,16]
  x_mid    = x + dx * (update_rand <= 0.5)
  life     = pre_life & (maxpool3x3(x_mid[...,3]) > 0.1)
  x_new    = x_mid * life
  returns (x_new, dx)

Mapping: 8 NeuronCores, pure data-parallel over batch (2 images/core).
The 3x3 conv + fc0 are fused into two matmuls (K=96 covering horizontal
taps 0+1 via a column-shifted duplicate of x on partitions 0-47, plus
K=48 for tap 2) with sobel coefficients folded into the weights on the
host.  fc1 runs operand-swapped (h stationary) so dx lands pixel-major.
The elementwise tail runs in a "pixel slab" layout [128 = w%128,
(img,row,half), c]; maxpool uses free-dim shifts vertically and PE
shift-permutation matmuls horizontally.
"""

import functools
import os
import sys

import numpy as np

_TRN_REPO = os.environ.get("TRN_RL_REPO", "/opt/trn_rl_repo")
if _TRN_REPO not in sys.path:
    sys.path.insert(0, _TRN_REPO)

import concourse.bass as bass
import concourse.bacc as bacc
import concourse.tile as tile
from concourse import mybir
from concourse.bass_utils import run_bass_kernel_spmd

F32 = mybir.dt.float32
F32R = mybir.dt.float32r
BF16 = mybir.dt.bfloat16
BF16_NP = mybir.dt.np(mybir.dt.bfloat16)

C = 16          # channels
HID = 128       # hidden dim
PW = 128        # partitions used as w-position within a half
N_CORES = 8
FIRE_RATE = 0.5
ALIVE_THR = 0.1

LAST_RESULTS = None  # BassKernelResults of the most recent kernel() call


# ---------------------------------------------------------------------------
# device program
# ---------------------------------------------------------------------------

def build_program(NI, H, W, TR=16):
    """Build the Bass program for one core processing NI images of HxW."""
    NH = W // PW                  # halves per row
    assert W % PW == 0 and H % TR == 0 and TR % 2 == 0
    Hp, Wp = H + 2, W + 2
    NRH = NI * H * NH             # total (img,row,half) count
    GRP = TR * NH * C             # psum free size per row-tile (=512 for TR=16)
    assert GRP <= 512

    nc = bacc.Bacc(trn_type="TRN2")

    # xq layout [NI, 96, H, Wp]: partition-major with uniform stride H*Wp.
    # Partitions 0-47 hold the column-shifted copy (tap1/tap2 windows),
    # 48-95 the unshifted rows (tap0); dy halo baked in per partition row.
    xh = nc.dram_tensor("xh", [NI * 96 * H, Wp], BF16, kind="ExternalInput")
    x_px = nc.dram_tensor("x_px", [PW, NRH, C], BF16, kind="ExternalInput")
    ur = nc.dram_tensor("ur", [PW, NRH], F32, kind="ExternalInput")
    B2d = nc.dram_tensor("B2d", [96, HID], BF16, kind="ExternalInput")
    B3d = nc.dram_tensor("B3d", [96, HID], BF16, kind="ExternalInput")
    w1d = nc.dram_tensor("w1d", [HID, C], BF16, kind="ExternalInput")
    SEd = nc.dram_tensor("SEd", [PW, PW], BF16, kind="ExternalInput")
    SWd = nc.dram_tensor("SWd", [PW, PW], BF16, kind="ExternalInput")
    dxo = nc.dram_tensor("dxo", [PW, NRH, C], BF16, kind="ExternalOutput")
    xno = nc.dram_tensor("xno", [PW, NRH, C], BF16, kind="ExternalOutput")

    with tile.TileContext(nc) as tc:
        _emit(tc, locals())
    nc.compile()
    return nc


def _emit(tc, t):
    nc = tc.nc
    NI, H, W, TR = t["NI"], t["H"], t["W"], t["TR"]
    NH, Hp, Wp, NRH, GRP = t["NH"], t["Hp"], t["Wp"], t["NRH"], t["GRP"]
    xh, x_px, ur, B2d, B3d, w1d, SEd, SWd, dxo, xno = (
        t["xh"], t["x_px"], t["ur"], t["B2d"], t["B3d"], t["w1d"],
        t["SEd"], t["SWd"], t["dxo"], t["xno"])
    TRNH = TR * NH
    AL = mybir.AluOpType

    from contextlib import ExitStack
    ctx = ExitStack()
    with ctx:
        singles = ctx.enter_context(tc.tile_pool(name="singles", bufs=1))
        xc_pool = ctx.enter_context(tc.tile_pool(name="xc", bufs=4))
        h_pool = ctx.enter_context(tc.tile_pool(name="h", bufs=8))
        st_pool = ctx.enter_context(tc.tile_pool(name="st", bufs=4))
        ps_h = ctx.enter_context(tc.tile_pool(name="ps_h", bufs=4, space="PSUM"))
        ps_dx = ctx.enter_context(tc.tile_pool(name="ps_dx", bufs=2, space="PSUM"))
        ps_scr = ctx.enter_context(tc.tile_pool(name="ps_scr", bufs=1, space="PSUM"))

        # ---- constants / weights ----
        # b2_sb rows 0-47 = tap dx=1 weights, rows 48-95 = tap dx=0 (matches
        # xc2's partition layout: shifted-copy block first).
        b2_sb = singles.tile([96, HID], BF16)
        nc.sync.dma_start(out=b2_sb, in_=B2d.ap())
        # b3 zero-padded to K=96 so both fc0 matmuls use the same (128,128)
        # PE tile config (a 64-row reconfig serializes LDWEIGHTS, +110ns/mm)
        b3_sb = singles.tile([96, HID], BF16)
        nc.sync.dma_start(out=b3_sb, in_=B3d.ap())
        w1_sb = singles.tile([HID, C], BF16)
        nc.sync.dma_start(out=w1_sb, in_=w1d.ap())
        se_sb = singles.tile([PW, PW], BF16)
        nc.sync.dma_start(out=se_sb, in_=SEd.ap())
        sw_sb = singles.tile([PW, PW], BF16)
        nc.sync.dma_start(out=sw_sb, in_=SWd.ap())
        # preload the ACT RELU table so tile 0's relus don't pay it
        wtab = singles.tile([PW, 2], BF16)
        nc.scalar.activation(out=wtab, in_=se_sb[:, 0:2],
                             func=mybir.ActivationFunctionType.Relu)

        # ---- update mask (DMA emitted inside the loop after tile 0's x
        # load so it doesn't delay the first matmul) ----
        ur_sb = singles.tile([PW, NRH], F32)
        um_sb = singles.tile([PW, NRH], F32)

        # ---- PE pre-sync dummies ----
        # Fused 4-byte-weight matmuls (f32/f32r) can carry only one sync
        # wait; touch each DMA-loaded operand once from PE so real matmuls
        # never need more than one.
        scr = ps_scr.tile([PW, 2], F32, tag="scr")
        nc.tensor.matmul(out=scr, lhsT=b2_sb[:], rhs=b2_sb[:, 0:2],
                         start=True, stop=True)
        nc.tensor.matmul(out=scr, lhsT=b3_sb[:], rhs=b3_sb[:, 0:2],
                         start=True, stop=True)
        scr2f = ps_scr.tile([PW, 2], F32, tag="scr")
        scr2 = scr2f[0:C, :]
        nc.tensor.matmul(out=scr2, lhsT=w1_sb[:], rhs=w1_sb[:, 0:2],
                         start=True, stop=True)

        # ---- residents ----
        xmid = singles.tile([PW, NRH, C], F32)
        ax = singles.tile([PW, NRH], BF16)   # alpha of x
        am = singles.tile([PW, NRH], BF16)   # alpha of x_mid

        # ---- mask scratch (bf16; shared across images/parts) ----
        HB = H * NH // 2                      # free size of one part
        vm = singles.tile([PW, NRH], BF16)
        vm2 = singles.tile([PW, NRH], BF16)
        m3 = singles.tile([PW, NRH], BF16)
        plx = singles.tile([PW, NRH], BF16)
        plm = singles.tile([PW, NRH], BF16)
        # cross-half neighbour staging: gather (DVE, strided->contig) into a
        # legal base partition, then a single-descriptor DMA moves it to the
        # partition where the patch max needs it; other partitions stay 0
        NR2 = H // 2
        sge = singles.tile([PW, NR2], BF16)
        nc.vector.memset(sge, 0.0)
        sgw = singles.tile([PW, NR2], BF16)
        nc.vector.memset(sgw, 0.0)

        def emit_masks_and_xnew(img, r0, r1):
            SI = img * H * NH
            S, E = SI + r0 * NH, SI + r1 * NH
            L = E - S

            for alpha, pl in ((ax, plx), (am, plm)):
                # vertical 3-max over rows [r0, r1) (clamped at image rows)
                if r0 == 0:
                    nc.vector.tensor_copy(out=vm[:, S:S + NH],
                                          in_=alpha[:, S:S + NH])
                    nc.vector.tensor_tensor(
                        out=vm[:, S + NH:E], in0=alpha[:, S + NH:E],
                        in1=alpha[:, S:E - NH], op=AL.max)
                else:
                    nc.vector.tensor_tensor(
                        out=vm[:, S:E], in0=alpha[:, S:E],
                        in1=alpha[:, S - NH:E - NH], op=AL.max)
                if r1 == H:
                    nc.vector.tensor_tensor(
                        out=vm2[:, S:E - NH], in0=vm[:, S:E - NH],
                        in1=alpha[:, S + NH:E], op=AL.max)
                    nc.vector.tensor_copy(out=vm2[:, E - NH:E],
                                          in_=vm[:, E - NH:E])
                else:
                    nc.vector.tensor_tensor(
                        out=vm2[:, S:E], in0=vm[:, S:E],
                        in1=alpha[:, S + NH:E + NH], op=AL.max)

                # horizontal 3-max: bf16 PE shift-permutation matmuls (exact
                # copy through the array; border partitions get psum 0)
                psf = ps_scr.tile([PW, 2, L], F32, tag="shift")
                nc.tensor.matmul(out=psf[:, 0, :], lhsT=se_sb[:],
                                 rhs=vm2[:, S:E], start=True, stop=True)
                nc.tensor.matmul(out=psf[:, 1, :], lhsT=sw_sb[:],
                                 rhs=vm2[:, S:E], start=True, stop=True)
                nc.vector.tensor_tensor(out=m3[:, S:E], in0=vm2[:, S:E],
                                        in1=psf[:, 0, :], op=AL.max)
                nc.vector.tensor_tensor(out=m3[:, S:E], in0=m3[:, S:E],
                                        in1=psf[:, 1, :], op=AL.max)
                # cross-half seams (NH=2): east neighbour of (p=127, hf=0)
                # is vm2[0, (r, 1)]; west neighbour of (p=0, hf=1) is
                # vm2[127, (r, 0)]
                def rh(t, p0, pn, hf):
                    return t[p0:p0 + pn, S:E].rearrange(
                        "p (r h) -> p r h", h=NH)[:, :, hf]

                NRr = r1 - r0
                nc.vector.tensor_copy(out=sge[0:1, 0:NRr],
                                      in_=rh(vm2, 0, 1, 1))
                nc.sync.dma_start(out=sge[127:128, 0:NRr],
                                  in_=sge[0:1, 0:NRr])
                nc.vector.tensor_tensor(
                    out=rh(m3, 96, 32, 0), in0=rh(m3, 96, 32, 0),
                    in1=sge[96:128, 0:NRr], op=AL.max)
                nc.vector.tensor_copy(out=sgw[96:128, 0:NRr],
                                      in_=rh(vm2, 96, 32, 0))
                nc.sync.dma_start(out=sgw[0:1, 0:NRr],
                                  in_=sgw[127:128, 0:NRr])
                nc.vector.tensor_tensor(
                    out=rh(m3, 0, 32, 1), in0=rh(m3, 0, 32, 1),
                    in1=sgw[0:32, 0:NRr], op=AL.max)
                nc.vector.tensor_scalar(
                    out=pl[:, S:E], in0=m3[:, S:E], scalar1=ALIVE_THR,
                    scalar2=None, op0=AL.is_gt)

            life = plx  # reuse: life = pre_life * mid_life
            nc.vector.tensor_tensor(out=life[:, S:E], in0=plx[:, S:E],
                                    in1=plm[:, S:E], op=AL.mult)
            for it in range(r0 // TR, r1 // TR):
                g0 = SI + it * TRNH
                xns = st_pool.tile([PW, TRNH, C], BF16, tag="xns")
                nc.vector.tensor_tensor(
                    out=xns, in0=xmid[:, g0:g0 + TRNH, :],
                    in1=life[:, g0:g0 + TRNH, None].to_broadcast([PW, TRNH, C]),
                    op=AL.mult)
                nc.sync.dma_start(out=xno.ap()[:, g0:g0 + TRNH, :], in_=xns,
                                  max_dma_last_dim=256)

        # ================= main matmul + dx/x_mid loop =================
        n_tiles = NI * (H // TR)
        for tt in range(n_tiles):
            img, a = divmod(tt, H // TR)
            a *= TR                       # first real row of this tile
            g0 = (img * H + a) * NH       # first rh index of this tile

            # XC2 partitions 0-47: x shifted one column left (padded col w+1
            # at stored col w); partitions 48-95: unshifted.  Gives taps 0+1
            # as a single K=96 matmul (window 0) and tap 2 as K=48 (window 1
            # into the shifted block) with rhs base partition 0.
            xc2 = xc_pool.tile([96, TR, Wp], BF16)
            # one call, uniform partition stride -> 384 ~2KB descriptors
            # spread across all 16 SDMA engines
            src = bass.AP(
                tensor=xh.ap().tensor,
                offset=(img * 96 * H + a) * Wp,
                ap=[[H * Wp, 96], [Wp, TR], [1, Wp]])
            nc.sync.dma_start(out=xc2, in_=src, max_dma_last_dim=1032)
            if tt == 0:
                nc.sync.dma_start(out=ur_sb, in_=ur.ap(),
                                  max_dma_last_dim=512)
                nc.vector.tensor_scalar(
                    out=um_sb, in0=ur_sb, scalar1=FIRE_RATE, scalar2=None,
                    op0=AL.is_le)

            ps2 = ps_dx.tile([PW, TRNH, C], F32)
            # emit fc0 matmuls pair-grouped by weight (b2 b2 b3 b3) so every
            # LDWEIGHTS has a full 512-cycle matmul to prefetch under, then
            # the relus, then the fc1 matmuls (LDWEIGHTS-bound)
            pshs, hsbs = [], []
            for q in range(TR // 4):
                pshA = ps_h.tile([HID, 2, W], F32, tag="psh0")
                pshB = ps_h.tile([HID, 2, W], F32, tag="psh0")
                pshs += [pshA, pshB]
                pA, pB = 4 * q, 4 * q + 2
                nc.tensor.matmul(
                    out=pshA, lhsT=b2_sb[:], rhs=xc2[0:96, pA:pA + 2, 0:W],
                    start=True, stop=False)
                nc.tensor.matmul(
                    out=pshB, lhsT=b2_sb[:], rhs=xc2[0:96, pB:pB + 2, 0:W],
                    start=True, stop=False)
                nc.tensor.matmul(
                    out=pshA, lhsT=b3_sb[:], rhs=xc2[0:96, pA:pA + 2, 1:1 + W],
                    start=False, stop=True)
                nc.tensor.matmul(
                    out=pshB, lhsT=b3_sb[:], rhs=xc2[0:96, pB:pB + 2, 1:1 + W],
                    start=False, stop=True)
            for p in range(TR // 2):
                hsb = h_pool.tile([HID, 2, W], BF16)
                hsbs.append(hsb)
                if p >= 6:
                    nc.vector.tensor_scalar(
                        out=hsb, in0=pshs[p], scalar1=0.0, scalar2=None,
                        op0=AL.max)
                else:
                    nc.scalar.activation(
                        out=hsb, in_=pshs[p],
                        func=mybir.ActivationFunctionType.Relu)
            for p in range(TR // 2):
                # fc1, operand-swapped: dx[pix, c] chunks
                for rp in range(2):
                    for hf in range(NH):
                        nc.tensor.matmul(
                            out=ps2[:, (2 * p + rp) * NH + hf, :],
                            lhsT=hsbs[p][:, rp, hf * PW:(hf + 1) * PW],
                            rhs=w1_sb[:],
                            start=True, stop=True)

            dxs = st_pool.tile([PW, TRNH, C], BF16, tag="dxs")
            nc.vector.tensor_copy(out=dxs, in_=ps2)
            nc.sync.dma_start(out=dxo.ap()[:, g0:g0 + TRNH, :], in_=dxs,
                              max_dma_last_dim=256)

            xps = st_pool.tile([PW, TRNH, C], BF16, tag="xps")
            nc.sync.dma_start(out=xps, in_=x_px.ap()[:, g0:g0 + TRNH, :],
                              max_dma_last_dim=256)
            nc.gpsimd.tensor_copy(out=ax[:, g0:g0 + TRNH], in_=xps[:, :, 3])

            # x_mid = x + dx * um  (gpsimd: keep DVE free for relu/copies)
            dxm = st_pool.tile([PW, TRNH, C], F32, tag="dxm")
            nc.gpsimd.tensor_tensor(
                out=dxm, in0=dxs,
                in1=um_sb[:, g0:g0 + TRNH, None].to_broadcast([PW, TRNH, C]),
                op=AL.mult)
            nc.gpsimd.tensor_tensor(
                out=xmid[:, g0:g0 + TRNH, :], in0=xps, in1=dxm, op=AL.add)
            nc.gpsimd.tensor_copy(
                out=am[:, g0:g0 + TRNH], in_=xmid[:, g0:g0 + TRNH, 3])

            QH = H // 4
            if a > 0 and a % QH == 0:
                # rows [a-QH, a) are maskable as soon as the alpha of row a
                # exists; overlaps the remaining matmul work
                emit_masks_and_xnew(img, a - QH, a)
            if a + TR == H:
                if img == NI - 1:
                    # last image: two smaller parts so the exposed tail
                    # (nothing left to overlap with) is as short as possible
                    emit_masks_and_xnew(img, H - QH, H - QH // 2)
                    emit_masks_and_xnew(img, H - QH // 2, H)
                else:
                    emit_masks_and_xnew(img, H - QH, H)


def _pslice(tile_, p, hf, NH, hrng):
    """[1, hrng] AP of tile_ at partition p, free elements hf::NH."""
    return tile_[p:p + 1, :].rearrange("p (r h) -> p r h", h=NH)[:, :, hf]


def _prange(tile_, p0, cnt, hf, NH, hrng):
    """[cnt, hrng] AP of tile_ at partitions [p0,p0+cnt), free elems hf::NH."""
    return tile_[p0:p0 + cnt, :].rearrange("p (r h) -> p r h", h=NH)[:, :, hf]


# ---------------------------------------------------------------------------
# host side
# ---------------------------------------------------------------------------

def _sobel():
    kx = np.outer([1.0, 2.0, 1.0], [-1.0, 0.0, 1.0]) / 8.0
    ky = kx.T
    return kx, ky


def make_weights(w0, w1):
    """Fold sobel taps into fc0 -> B2[96,128] = [tap1; tap0], B3[48,128]."""
    kx, ky = _sobel()
    w0 = np.asarray(w0, np.float32)         # [48, 128]
    W0x = w0[0::3]                           # [16, 128]
    W0gx = w0[1::3]
    W0gy = w0[2::3]
    Bw = np.zeros((3, 48, HID), np.float32)  # cast to bf16 at return
    for dy in range(3):
        for dxi in range(3):
            m = kx[dy, dxi] * W0gx + ky[dy, dxi] * W0gy
            if dy == 1 and dxi == 1:
                m = m + W0x
            Bw[dxi, dy * C:(dy + 1) * C, :] = m
    B2 = np.concatenate([Bw[1], Bw[0]], axis=0)
    B3 = np.concatenate([Bw[2], np.zeros((48, HID), np.float32)], axis=0)
    return (B2.astype(BF16_NP), B3.astype(BF16_NP),
            np.asarray(w1, BF16_NP))


def host_inputs(x_core, ur_core, B2, B3, w1, H, W):
    """Build the per-core input map from [NI,H,W,C] x and [NI,H,W,1] rand."""
    NI = x_core.shape[0]
    NH = W // PW
    Hp, Wp = H + 2, W + 2
    # xq[i, dy*16+c + 48, r, :] = xpad[i, c, r+dy, :]; partitions 0-47 are
    # the same rows shifted one column left (zero fill)
    xpad = np.zeros((NI, C, Hp, Wp), BF16_NP)
    xpad[:, :, 1:H + 1, 1:W + 1] = x_core.transpose(0, 3, 1, 2)
    xh = np.zeros((NI, 96, H, Wp), BF16_NP)
    for dy in range(3):
        blk = xpad[:, :, dy:dy + H, :]
        xh[:, 48 + dy * C:48 + (dy + 1) * C] = blk
        xh[:, dy * C:(dy + 1) * C, :, 0:Wp - 1] = blk[..., 1:]
    x_px = np.ascontiguousarray(
        x_core.reshape(NI, H, NH, PW, C).transpose(3, 0, 1, 2, 4)
    ).reshape(PW, NI * H * NH, C).astype(BF16_NP)
    urp = np.ascontiguousarray(
        ur_core[..., 0].reshape(NI, H, NH, PW).transpose(3, 0, 1, 2)
    ).reshape(PW, NI * H * NH)
    return {
        "xh": xh.reshape(NI * 96 * H, Wp),
        "x_px": x_px,
        "ur": urp,
        "B2d": B2,
        "B3d": B3,
        "w1d": w1,
        "SEd": np.eye(PW, k=-1, dtype=np.float32).astype(BF16_NP),
        "SWd": np.eye(PW, k=1, dtype=np.float32).astype(BF16_NP),
    }


def unpack_output(dev, NI, H, W):
    """[PW, NRH, C] device layout -> [NI, H, W, C] float32."""
    NH = W // PW
    return np.ascontiguousarray(
        np.asarray(dev, np.float32).reshape(PW, NI, H, NH, C)
        .transpose(1, 2, 3, 0, 4)
    ).reshape(NI, H, W, C)


@functools.lru_cache(maxsize=2)
def _cached_program(NI, H, W, TR):
    return build_program(NI, H, W, TR=TR)


def kernel(x, update_rand, w0, w1):
    x = np.asarray(x, np.float32)
    update_rand = np.asarray(update_rand, np.float32)
    B, H, W, _ = x.shape
    NI = B // N_CORES
    B2, B3, w1f = make_weights(w0, w1)

    nc = _cached_program(NI, H, W, 16)
    in_maps = [
        host_inputs(x[i * NI:(i + 1) * NI], update_rand[i * NI:(i + 1) * NI],
                    B2, B3, w1f, H, W)
        for i in range(N_CORES)
    ]
    res = run_bass_kernel_spmd(nc, in_maps, core_ids=list(range(N_CORES)))
    global LAST_RESULTS
    LAST_RESULTS = res
    x_new = np.concatenate(
        [unpack_output(r["xno"], NI, H, W) for r in res.results], axis=0)
    dx = np.concatenate(
        [unpack_output(r["dxo"], NI, H, W) for r in res.results], axis=0)
    return x_new, dx

